# revision 1
# baseline (speedup 1.0000x reference)
"""Trainium2 Bass kernel for nn_ASSC_66657892434080.

Reference computation (per batch sample b, data-parallel over 8 cores):
    q = wq @ x_1[b] + bq ; k = wk @ x[b] + bk          (1x1 convs)
    proj_query = PSP(q) [256,280] ; proj_key = PSP(k) [32,280]
    aff = sigmoid(proj_query @ proj_key^T)             [256,32]
    agg_w = (aff @ con.reshape(32, 256*9))             per-sample 3x3 weights
    out[b] = conv3x3(x_1[b], agg_w)                    (grouped conv, groups=B)

Key restructuring (validated vs reference in numpy to ~1e-6 rel):
  * PSP (adaptive-avg-pool pyramid) is linear, so PSP(wq@x+bq) = wq@PSP(x)+bq.
    The full-res q/k tensors are never materialized.
  * PSP itself = cumulative-sum along W, bin-differencing, cumsum along H,
    bin-differencing -> all on the Vector engine.
  * The grouped 3x3 conv = 9 shifted matmuls accumulating in PSUM over
    (tap, cin-chunk); contraction over cin=128/chunk on the partition dim.
  * con is host-reordered to [32, tap*256+cin] so the synthesized weights
    come out directly as the matmul lhsT tiles [cin, cout] per tap.
"""

import numpy as np
import concourse.bass as bass
import concourse.bacc as bacc
import concourse.tile as tile
import concourse.mybir as mybir
import bass_rust
from concourse.bass_utils import run_bass_kernel_spmd

B, C, H, W = 8, 256, 96, 96
C8 = 32
HW = H * W                      # 9216
POOL_SIZES = (1, 3, 5, 7, 14)   # -> 30 1-D bins, 280 2-D positions
NB = sum(POOL_SIZES)            # 30
NP = sum(s * s for s in POOL_SIZES)  # 280
STRIP = 96                      # pooling strip rows (full chunk)
NSTRIP = H // STRIP             # 3
ROWS_PER_SCHUNK = 4             # conv output rows per PSUM chunk
NSCHUNK = H // ROWS_PER_SCHUNK  # 24
SCHUNK = ROWS_PER_SCHUNK * W    # 384
F32 = mybir.dt.float32
F32R = mybir.dt.float32r
BF16 = mybir.dt.bfloat16


def _pool_bins(n, s):
    return [((i * n) // s, -((-(i + 1) * n) // s)) for i in range(s)]


WBINS = [b for s in POOL_SIZES for b in _pool_bins(W, s)]   # 30 (ws, we)
HBINS = {s: _pool_bins(H, s) for s in POOL_SIZES}
JBASE = {}
B280 = {}
_j = _p = 0
for _s in POOL_SIZES:
    JBASE[_s] = _j
    B280[_s] = _p
    _j += _s
    _p += _s * _s


def _pool_indicator():
    """Mk [9216, 280] bf16: 1.0 where spatial (h,w) falls in pooled bin p."""
    import ml_dtypes
    Mk = np.zeros((H * W, NP), np.float32)
    for s in POOL_SIZES:
        hb, wb = _pool_bins(H, s), _pool_bins(W, s)
        for o, (hs, he) in enumerate(hb):
            for p, (ws, we) in enumerate(wb):
                col = B280[s] + o * s + p
                for h in range(hs, he):
                    Mk[h * W + ws:h * W + we, col] = 1.0
    return Mk.astype(ml_dtypes.bfloat16)


def _area_inv():
    ai = np.zeros(NP, np.float32)
    for s in POOL_SIZES:
        hb, wb = _pool_bins(H, s), _pool_bins(W, s)
        for o, (hs, he) in enumerate(hb):
            for p, (ws, we) in enumerate(wb):
                ai[B280[s] + o * s + p] = 1.0 / ((he - hs) * (we - ws))
    return ai


def _split_multiwait_ctrl(nc, default_limit=1):
    """walrus in this container rejects instructions carrying more than one
    sem wait; move extras onto preceding same-engine drains (the engine
    blocks on those first, preserving semantics).  NEVER split PE
    instructions: the PE queue is a reorder window that pulls LDWEIGHTS
    ahead of in-flight work, so a wait moved onto a separate drain no
    longer gates the next matmul's weight load (observed as stale-weight
    garbage on HW).  Matmult accepts multiple waits in this walrus."""
    for f in nc.m.functions:
        for bb in f.blocks:
            new_list = []
            for inst in bb.instructions:
                si = inst.sync_info
                waits = list(si.on_wait) if si and si.on_wait else []
                mw = default_limit
                if getattr(inst, "engine", None) == mybir.EngineType.PE:
                    mw = 99
                if len(waits) > mw:
                    for k, w in enumerate(waits[:-mw]):
                        pre = mybir.InstDrain(name=f"{inst.name}-w{k}", ins=[], outs=[])
                        pre.engine = inst.engine
                        pre.sync_info = bass_rust.SyncInfo(on_wait=[w], on_update=[])
                        new_list.append(pre)
                    inst.sync_info = bass_rust.SyncInfo(
                        on_wait=waits[-mw:],
                        on_update=list(si.on_update) if si.on_update else [],
                    )
                new_list.append(inst)
            bb.instructions[:] = new_list


def _sv(ap2d, dims):
    """Custom strided view: keep the partition dim of a 2D AP, replace the
    free dims with explicit (step, count) pairs."""
    import dataclasses
    return dataclasses.replace(ap2d, ap=[list(ap2d.ap[0])] + [[s, c] for s, c in dims])


def _uniform_runs(bins):
    """Group consecutive bins into runs where both the start and end
    boundaries advance with constant stride -> one strided instruction per
    run."""
    runs = []
    i = 0
    n = len(bins)
    while i < n:
        if i == n - 1:
            runs.append((i, 1, 0, 0))
            i += 1
            continue
        ds = bins[i + 1][0] - bins[i][0]
        de = bins[i + 1][1] - bins[i][1]
        j = i + 1
        while (j + 1 < n and bins[j + 1][0] - bins[j][0] == ds
               and bins[j + 1][1] - bins[j][1] == de):
            j += 1
        runs.append((i, j - i + 1, ds, de))
        i = j + 1
    return runs


def _emit_psp(nc, P_out, src2d, F1, G, F2, nstrip=1):
    """Pool one 128-channel chunk into P_out [128, 280] (raw bin SUMS).

    src2d: callable strip -> (flat 2D AP covering rows [strip*32, +32),
    rowlen, col_off) where element (r, w) of the strip lives at flat index
    r*rowlen + col_off + w.  Extra (zero-pad) columns between rows are
    harmless: the cumsum carries through them and bin differences stay
    within-row.  G layout: [128, NB*H] flat j*96+h (j = 1-D W-bin index).
    """
    add, byp, = mybir.AluOpType.add, mybir.AluOpType.bypass
    rows = H // nstrip
    for strip in range(nstrip):
        h0 = strip * rows
        src, rowlen, coff = src2d(strip)
        n = rows * rowlen
        nc.vector.memset(F1[:, 0:1], 0.0)
        nc.vector.tensor_tensor_scan(F1[:, 1:1 + n], src, _sv(F1[:, 0:1], [(0, n)]),
                                     0.0, add, byp)
        # F1 col (r*rowlen + coff + w) = cumsum through (r, w-1); one
        # strided instruction per uniform run of bins
        jbase = 0
        for s in POOL_SIZES:
            sb = _pool_bins(W, s)
            for (i0, cnt, ds, de) in _uniform_runs(sb):
                ws, we = sb[i0]
                j0 = jbase + i0
                nc.vector.tensor_sub(
                    _sv(G[:, j0 * H + h0:], [(H, cnt), (1, rows)]),
                    _sv(F1[:, coff + we:], [(de, cnt), (rowlen, rows)]),
                    _sv(F1[:, coff + ws:], [(ds, cnt), (rowlen, rows)]),
                )
            jbase += s
    nc.vector.memset(F2[:, 0:1], 0.0)
    nc.vector.tensor_tensor_scan(F2[:, 1:1 + NB * H], G[:, :],
                                 _sv(F2[:, 0:1], [(0, NB * H)]), 0.0, add, byp)
    for s in POOL_SIZES:
        jb = JBASE[s]
        hb = HBINS[s]
        for (o0, cnt, ds, de) in _uniform_runs(hb):
            hs, he = hb[o0]
            nc.vector.tensor_sub(
                _sv(P_out[:, B280[s] + o0 * s:], [(s, cnt), (1, s)]),
                _sv(F2[:, jb * H + he:], [(de, cnt), (H, s)]),
                _sv(F2[:, jb * H + hs:], [(ds, cnt), (H, s)]),
            )




def _fix_ldweights_waits(nc):
    """Tile legalization splits 2-byte matmuls into standalone InstLdweights +
    InstMatmult, but can leave the wait that gates the WEIGHT data on the
    matmul -- after the weights were already latched -> stale-weight races on
    HW.  Move every wait of the pair onto the ldweights (waiting earlier is
    always safe; LDW-LDW order is preserved by the PE queue).  walrus allows
    only one wait per ldweights, so extra waits become duplicated ldweights
    (reloading the same weights is idempotent)."""
    import copy
    for f in nc.m.functions:
        for bb in f.blocks:
            insts = bb.instructions
            new_list = []
            i = 0
            while i < len(insts):
                inst = insts[i]
                nxt = insts[i + 1] if i + 1 < len(insts) else None
                if (type(inst).__name__ == "InstLdweights" and nxt is not None
                        and type(nxt).__name__ == "InstMatmult"):
                    wl = list(inst.sync_info.on_wait) if inst.sync_info and inst.sync_info.on_wait else []
                    wm = list(nxt.sync_info.on_wait) if nxt.sync_info and nxt.sync_info.on_wait else []
                    waits = wl + wm
                    mm_upd = list(nxt.sync_info.on_update) if nxt.sync_info and nxt.sync_info.on_update else []
                    ld_upd = list(inst.sync_info.on_update) if inst.sync_info and inst.sync_info.on_update else []
                    if len(waits) > 1:
                        for k, w in enumerate(waits[:-1]):
                            pre = copy.deepcopy(inst)
                            pre.name = f"{inst.name}-ldw{k}"
                            pre.sync_info = bass_rust.SyncInfo(on_wait=[w], on_update=[])
                            new_list.append(pre)
                        inst.sync_info = bass_rust.SyncInfo(on_wait=[waits[-1]], on_update=ld_upd)
                        nxt.sync_info = bass_rust.SyncInfo(on_wait=[], on_update=mm_upd)
                    elif len(waits) == 1:
                        inst.sync_info = bass_rust.SyncInfo(on_wait=[waits[0]], on_update=ld_upd)
                        nxt.sync_info = bass_rust.SyncInfo(on_wait=[], on_update=mm_upd)
                    new_list.append(inst)
                    new_list.append(nxt)
                    i += 2
                    continue
                new_list.append(inst)
                i += 1
            bb.instructions[:] = new_list


def build_kernel(conv_dtype=BF16, split_ctrl=True, debug_taps=False):
    nc = bacc.Bacc("TRN2", target_bir_lowering=False, debug=False)

    x1 = nc.dram_tensor("x1", [2, 128, HW], conv_dtype, kind="ExternalInput")
    xx = nc.dram_tensor("xx", [2, 128, HW], BF16, kind="ExternalInput")
    wqT = nc.dram_tensor("wqT", [2, 128, C], F32, kind="ExternalInput")
    wkT = nc.dram_tensor("wkT", [2, 128, C8], F32, kind="ExternalInput")
    bqb = nc.dram_tensor("bqb", [128, C], F32, kind="ExternalInput")
    bkb = nc.dram_tensor("bkb", [128, C8], F32, kind="ExternalInput")
    conr = nc.dram_tensor("conr", [C8, 9 * C], conv_dtype, kind="ExternalInput")
    ainv = nc.dram_tensor("ainv", [128, NP], F32, kind="ExternalInput")
    out = nc.dram_tensor("out", [2, 128, HW], F32, kind="ExternalOutput")
    dbg = None
    if debug_taps:
        dbg = {
            "dbg_Pq0": nc.dram_tensor("dbg_Pq0", [128, NP], F32, kind="ExternalOutput"),
            "dbg_Pk0": nc.dram_tensor("dbg_Pk0", [128, NP], F32, kind="ExternalOutput"),
            "dbg_affT": nc.dram_tensor("dbg_affT", [C8, C], F32, kind="ExternalOutput"),
            "dbg_wsb": nc.dram_tensor("dbg_wsb", [128, 18 * C], F32, kind="ExternalOutput"),
            "dbg_xpad": nc.dram_tensor("dbg_xpad", [128, (H + 2) * (W + 2)], F32, kind="ExternalOutput"),
        }

    with tile.TileContext(nc) as tc:
        with (
            tc.tile_pool(name="consts", bufs=1) as cpool,
            tc.tile_pool(name="xpool", bufs=1) as xpool,
            tc.tile_pool(name="scratch", bufs=1) as spool,
            tc.tile_pool(name="wstage", bufs=2) as wpool,
            tc.tile_pool(name="ostage", bufs=4) as opool,
        ):
            # ---- constants ----
            wq_t = [cpool.tile([128, C], F32, tag=f"wq{i}", name=f"wq{i}") for i in range(2)]
            wk_t = [cpool.tile([128, C8], F32, tag=f"wk{i}", name=f"wk{i}") for i in range(2)]
            bq_t = cpool.tile([128, C], F32, tag="bq", name="bq")
            bk_t = cpool.tile([128, C8], F32, tag="bk", name="bk")
            ai_t = cpool.tile([128, NP], F32, tag="ai", name="ai")
            for i in range(2):
                nc.sync.dma_start(wq_t[i][:], wqT.ap()[i])
                nc.sync.dma_start(wk_t[i][:], wkT.ap()[i])
            nc.sync.dma_start(bq_t[:], bqb.ap())
            nc.sync.dma_start(bk_t[:], bkb.ap())
            nc.sync.dma_start(ai_t[:], ainv.ap())

            # ---- x_1 into zero-padded [128, 98, 98] tiles ----
            xpad = [xpool.tile([128, H + 2, W + 2], conv_dtype, tag=f"xpad{i}", name=f"xpad{i}") for i in range(2)]
            for cc in range(2):
                nc.vector.memset(xpad[cc][:, 0:H + 2:H + 1, :], 0.0)       # rows 0, 97
                nc.vector.memset(xpad[cc][:, 1:H + 1, 0:W + 2:W + 1], 0.0)  # cols 0, 97
            x1v = [x1.ap()[cc].rearrange("p (h w) -> p h w", w=W) for cc in range(2)]
            def load_xpad(cc, nrows=24):
                for h0 in range(0, H, nrows):
                    nc.sync.dma_start(
                        xpad[cc][:, 1 + h0:1 + h0 + nrows, 1:1 + W],
                        x1v[cc][:, h0:h0 + nrows, :],
                    )

            # ---- pooling scratch (DVE cumsum + bin differencing) ----
            xbuf = spool.tile([128, HW], BF16, tag="xbuf", name="xbuf")
            F1 = spool.tile([128, STRIP * (W + 2) + 1], F32, tag="F1", name="F1")
            G = spool.tile([128, NB * H], F32, tag="G", name="G")
            F2 = spool.tile([128, NB * H + 1], F32, tag="F2", name="F2")
            Pq = [cpool.tile([128, NP], F32, tag=f"Pq{i}", name=f"Pq{i}") for i in range(2)]
            Pk = [cpool.tile([128, NP], F32, tag=f"Pk{i}", name=f"Pk{i}") for i in range(2)]

            # pool x (k-side) chunk by chunk through xbuf; x_1 (xpad) loads
            # are interleaved behind the first xx chunk so pooling starts ASAP
            HWH = HW // 2
            def src_xx0(strip):
                # first chunk in two halves so the first scan starts sooner
                nc.sync.dma_start(xbuf[:, strip * HWH:(strip + 1) * HWH],
                                  xx.ap()[0][:, strip * HWH:(strip + 1) * HWH])
                if strip == 0:
                    load_xpad(0)
                    load_xpad(1)
                return xbuf[:, strip * HWH:(strip + 1) * HWH], W, 0
            def src_xx1(strip):
                nc.sync.dma_start(xbuf[:], xx.ap()[1])
                return xbuf[:], W, 0
            _emit_psp(nc, Pk[0], src_xx0, F1, G, F2, nstrip=2)
            _emit_psp(nc, Pk[1], src_xx1, F1, G, F2)
            # pool x_1 (q-side) straight from the padded tiles: rows are a
            # contiguous [96, 98] window; pad zeros flow through the cumsum
            xpflat = [xpad[cc][:].rearrange("p h w -> p (h w)") for cc in range(2)]
            for cc in range(2):
                def src_x1(strip, _cc=cc):
                    return (xpflat[_cc][:, (W + 2):(1 + H) * (W + 2)], W + 2, 1)
                _emit_psp(nc, Pq[cc], src_x1, F1, G, F2)

            # raw sums -> averages
            for cc in range(2):
                nc.vector.tensor_mul(Pq[cc][:], Pq[cc][:], ai_t[:])
                nc.vector.tensor_mul(Pk[cc][:], Pk[cc][:], ai_t[:])

            # ---- projections / affinity / weight synthesis ----
            PCH = [(0, 128), (128, 128), (256, 24)]
            pqT = [cpool.tile([n, C], F32, tag=f"pqT{i}", name=f"pqT{i}") for i, (_, n) in enumerate(PCH)]
            pkT = [cpool.tile([n, C8], F32, tag=f"pkT{i}", name=f"pkT{i}") for i, (_, n) in enumerate(PCH)]
            affT = cpool.tile([C8, C], conv_dtype, tag="affT", name="affT")
            w_sb = cpool.tile([128, 18 * C], conv_dtype, tag="w_sb", name="w_sb")

            with tc.tile_pool(name="psmall", bufs=2, space="PSUM") as pps:
                for i, (p0, n) in enumerate(PCH):
                    ps = pps.tile([n, C], F32, tag="ps", name="ps")
                    for cc in range(2):
                        nc.tensor.matmul(ps[:], Pq[cc][:, p0:p0 + n], wq_t[cc][:],
                                         start=(cc == 0), stop=(cc == 1))
                    nc.vector.tensor_add(pqT[i][:], ps[:], bq_t[:n, :])
                for i, (p0, n) in enumerate(PCH):
                    ps2 = pps.tile([n, C8], F32, tag="ps2", name="ps2")
                    for cc in range(2):
                        nc.tensor.matmul(ps2[:], Pk[cc][:, p0:p0 + n], wk_t[cc][:],
                                         start=(cc == 0), stop=(cc == 1))
                    nc.vector.tensor_add(pkT[i][:], ps2[:], bk_t[:n, :])
                pa = pps.tile([C8, C], F32, tag="pa", name="pa")
                for i in range(3):
                    nc.tensor.matmul(pa[:], pkT[i][:], pqT[i][:],
                                     start=(i == 0), stop=(i == 2))
                nc.scalar.activation(affT[:], pa[:], mybir.ActivationFunctionType.Sigmoid)
                for wc in range(18):
                    ct = wpool.tile([C8, 128], conv_dtype, tag="conr", name="conr")
                    nc.sync.dma_start(ct[:], conr.ap()[:, wc * 128:(wc + 1) * 128])
                    pw = pps.tile([128, C], F32, tag="pw", name="pw")
                    nc.tensor.matmul(pw[:], ct[:], affT[:],
                                     start=True, stop=True)
                    nc.scalar.copy(w_sb[:, wc * C:(wc + 1) * C], pw[:])

            if dbg is not None:
                dbg_f32 = cpool.tile([128, 18 * C], F32, tag="dbgf", name="dbgf")
                nc.sync.dma_start(dbg["dbg_Pq0"].ap(), Pq[0][:])
                nc.sync.dma_start(dbg["dbg_Pk0"].ap(), Pk[0][:])
                nc.vector.tensor_copy(dbg_f32[:C8, :C], affT[:])
                nc.sync.dma_start(dbg["dbg_affT"].ap(), dbg_f32[:C8, :C])
                nc.vector.tensor_copy(dbg_f32[:], w_sb[:])
                nc.sync.dma_start(dbg["dbg_wsb"].ap(), dbg_f32[:])
                dbg_xp = cpool.tile([128, (H + 2) * (W + 2)], F32, tag="dbgx", name="dbgx")
                nc.vector.tensor_copy(dbg_xp[:], xpad[0][:].rearrange("p h w -> p (h w)"))
                nc.sync.dma_start(dbg["dbg_xpad"].ap(), dbg_xp[:])

            # ---- the 3x3 grouped conv: 9 taps x 2 cin-chunks accumulate ----
            TAPS = [(t, cinc) for t in range(9) for cinc in range(2)]
            GRP = 4
            RS = 5
            CHUNKS = [(RS * s, RS) for s in range(H // RS)] + [(H - H % RS, H % RS)]
            with tc.tile_pool(name="pconv", bufs=2, space="PSUM") as ppc:
                for coutc in range(2):
                    for grp in range(len(CHUNKS) // GRP):
                        cgrp = CHUNKS[grp * GRP:(grp + 1) * GRP]
                        pts = [ppc.tile([128, RS * W], F32, tag=f"cv{i}", name=f"cv{i}")
                               for i in range(GRP)]
                        for ti, (t, cinc) in enumerate(TAPS):
                            dy, dx = t // 3, t % 3
                            wsl = w_sb[:, (t * 2 + cinc) * C + coutc * 128:
                                       (t * 2 + cinc) * C + coutc * 128 + 128]
                            for i, (r0, nr) in enumerate(cgrp):
                                rhs = xpad[cinc][:, r0 + dy:r0 + dy + nr, dx:dx + W]
                                nc.tensor.matmul(pts[i][:, :nr * W], wsl, rhs,
                                                 start=(ti == 0), stop=(ti == 17))
                        for i, (r0, nr) in enumerate(cgrp):
                            ot = opool.tile([128, RS * W], F32, tag="ot", name="ot")
                            nc.scalar.copy(ot[:, :nr * W], pts[i][:, :nr * W])
                            nc.sync.dma_start(
                                out.ap()[coutc][:, r0 * W:(r0 + nr) * W], ot[:, :nr * W])

    if split_ctrl:
        nc.compile()
        _fix_ldweights_waits(nc)
    return nc


_NC_CACHE = {}


def _get_nc():
    if "nc" not in _NC_CACHE:
        _NC_CACHE["nc"] = build_kernel()
    return _NC_CACHE["nc"]


def _tf32_round(x):
    u = np.ascontiguousarray(x, np.float32).view(np.uint32)
    u = (u + 0x0FFF + ((u >> 13) & 1)) & np.uint32(0xFFFFE000)
    return u.view(np.float32)


def _conv_cast(x):
    import ml_dtypes
    return np.ascontiguousarray(x, np.float32).astype(ml_dtypes.bfloat16)


def kernel(x_1, x, wq, bq, wk, bk, con):
    x_1 = _conv_cast(x_1)
    con = _conv_cast(con)
    x = _conv_cast(x)
    wq = np.asarray(wq, np.float32)
    bq = np.asarray(bq, np.float32)
    wk = np.asarray(wk, np.float32)
    bk = np.asarray(bk, np.float32)

    wqT_h = np.ascontiguousarray(wq.T).reshape(2, 128, C)
    wkT_h = np.ascontiguousarray(wk.T).reshape(2, 128, C8)
    bqb_h = np.tile(bq, (128, 1))
    bkb_h = np.tile(bk, (128, 1))
    # conr[k, tap*C + cin] = con[k, cin, dy, dx], tap = dy*3+dx
    conr_h = np.ascontiguousarray(con.transpose(2, 3, 1, 0).reshape(9 * C, C8).T)
    ainv_h = np.tile(_area_inv(), (128, 1))

    in_maps = []
    for b in range(B):
        in_maps.append({
            "x1": x_1[b].reshape(2, 128, HW),
            "xx": x[b].reshape(2, 128, HW),
            "wqT": wqT_h, "wkT": wkT_h, "bqb": bqb_h, "bkb": bkb_h,
            "conr": conr_h, "ainv": ainv_h,
        })
    global _last_in_maps
    _last_in_maps = in_maps
    nc = _get_nc()
    res = run_bass_kernel_spmd(nc, in_maps, list(range(B)))
    return np.stack([res.results[b]["out"].reshape(C, H, W) for b in range(B)])



# revision 7
# speedup vs baseline: 1.6834x; 1.6834x over previous
"""Trainium2 Bass kernel for nn_ASSC_66657892434080.

Reference computation (per batch sample b, data-parallel over 8 cores):
    q = wq @ x_1[b] + bq ; k = wk @ x[b] + bk          (1x1 convs)
    proj_query = PSP(q) [256,280] ; proj_key = PSP(k) [32,280]
    aff = sigmoid(proj_query @ proj_key^T)             [256,32]
    agg_w = (aff @ con.reshape(32, 256*9))             per-sample 3x3 weights
    out[b] = conv3x3(x_1[b], agg_w)                    (grouped conv, groups=B)

Restructuring (validated vs reference in numpy to ~5e-7):
  * conv(x, aff@con) == aff @ conv(x, con): the big per-sample conv becomes a
    32-channel conv z = conv3x3(x_1, con) plus a small matmul out = aff @ z.
    PE columns drop ~4.7x vs synthesizing [256,256,3,3] weights.
  * z computed as zpart[(dy,k), h, w] (dy packed into PE output columns, M=96)
    from 6 accumulating matmuls (3 dx shifts x 2 cin chunks); the dy row-shift
    is applied during the PSUM->SBUF (bf16) copy; out = affrep(96) @ zs.
  * PSP is linear: PSP(wq@x+bq) = wq@PSP(x)+bq (q side pools x_1 directly).
  * K side projects FIRST (k = wk@x on PE, 32 ch), k is repacked into thirds
    [(third,kk)=96 partitions, 32h x 96w] via matmul tile_position, so the
    pooling scans are 3x shorter; third-local bin sums are combined by
    accumulating identity-matmul transposes into pkT [p,32] tiles.
  * Output DMA'd in bf16 and upcast on host (rel-err budget 2e-2).
"""

import numpy as np
import concourse.bass as bass
import concourse.bacc as bacc
import concourse.tile as tile
import concourse.mybir as mybir
import bass_rust
from concourse.bass_utils import run_bass_kernel_spmd

B, C, H, W = 8, 256, 96, 96
C8 = 32
HW = H * W                      # 9216
POOL_SIZES = (1, 3, 5, 7, 14)   # -> 30 1-D bins, 280 2-D positions
NB = sum(POOL_SIZES)            # 30
NP = sum(s * s for s in POOL_SIZES)  # 280
STRIP = 96                      # q-side pooling strip rows (full chunk)
TH = 32                         # k-side rows per third
ROWS = 4                        # conv rows per PSUM chunk
NCH = H // ROWS                 # 24 chunks
F32 = mybir.dt.float32
BF16 = mybir.dt.bfloat16


def _pool_bins(n, s):
    return [((i * n) // s, -((-(i + 1) * n) // s)) for i in range(s)]


HBINS = {s: _pool_bins(H, s) for s in POOL_SIZES}
JBASE = {}
B280 = {}
_j = _p = 0
for _s in POOL_SIZES:
    JBASE[_s] = _j
    B280[_s] = _p
    _j += _s
    _p += _s * _s


def _area_inv():
    ai = np.zeros(NP, np.float32)
    for s in POOL_SIZES:
        hb, wb = _pool_bins(H, s), _pool_bins(W, s)
        for o, (hs, he) in enumerate(hb):
            for p, (ws, we) in enumerate(wb):
                ai[B280[s] + o * s + p] = 1.0 / ((he - hs) * (we - ws))
    return ai


def _split_multiwait_ctrl(nc, default_limit=1):
    """walrus in this container rejects instructions carrying more than one
    sem wait; move extras onto preceding same-engine drains.  NEVER split PE
    instructions (reorder window pulls LDWEIGHTS ahead)."""
    for f in nc.m.functions:
        for bb in f.blocks:
            new_list = []
            for inst in bb.instructions:
                si = inst.sync_info
                waits = list(si.on_wait) if si and si.on_wait else []
                mw = default_limit
                if getattr(inst, "engine", None) == mybir.EngineType.PE:
                    mw = 99
                if len(waits) > mw:
                    for k, w in enumerate(waits[:-mw]):
                        pre = mybir.InstDrain(name=f"{inst.name}-w{k}", ins=[], outs=[])
                        pre.engine = inst.engine
                        pre.sync_info = bass_rust.SyncInfo(on_wait=[w], on_update=[])
                        new_list.append(pre)
                    inst.sync_info = bass_rust.SyncInfo(
                        on_wait=waits[-mw:],
                        on_update=list(si.on_update) if si.on_update else [],
                    )
                new_list.append(inst)
            bb.instructions[:] = new_list


def _sv(ap2d, dims):
    """Strided view: keep partition dim, replace free dims with (step, count)."""
    import dataclasses
    return dataclasses.replace(ap2d, ap=[list(ap2d.ap[0])] + [[s, c] for s, c in dims])


def _uniform_runs(bins):
    """Group consecutive bins into runs with constant boundary strides."""
    runs = []
    i = 0
    n = len(bins)
    while i < n:
        if i == n - 1:
            runs.append((i, 1, 0, 0))
            i += 1
            continue
        ds = bins[i + 1][0] - bins[i][0]
        de = bins[i + 1][1] - bins[i][1]
        j = i + 1
        while (j + 1 < n and bins[j + 1][0] - bins[j][0] == ds
               and bins[j + 1][1] - bins[j][1] == de):
            j += 1
        runs.append((i, j - i + 1, ds, de))
        i = j + 1
    return runs


def _emit_psp(nc, P_out, src2d, F1, G, F2, nstrip=1):
    """Q-side: pool one 128-channel chunk into P_out [128, 280] (raw SUMS).
    G layout: [128, NB*H] flat j*96+h."""
    add, byp = mybir.AluOpType.add, mybir.AluOpType.bypass
    rows = H // nstrip
    for strip in range(nstrip):
        h0 = strip * rows
        src, rowlen, coff = src2d(strip)
        n = rows * rowlen
        nc.vector.memset(F1[:, 0:1], 0.0)
        nc.vector.tensor_tensor_scan(F1[:, 1:1 + n], src, _sv(F1[:, 0:1], [(0, n)]),
                                     0.0, add, byp)
        jbase = 0
        for s in POOL_SIZES:
            sb = _pool_bins(W, s)
            for (i0, cnt, ds, de) in _uniform_runs(sb):
                ws, we = sb[i0]
                j0 = jbase + i0
                nc.vector.tensor_sub(
                    _sv(G[:, j0 * H + h0:], [(H, cnt), (1, rows)]),
                    _sv(F1[:, coff + we:], [(de, cnt), (rowlen, rows)]),
                    _sv(F1[:, coff + ws:], [(ds, cnt), (rowlen, rows)]),
                )
            jbase += s
    nc.vector.memset(F2[:, 0:1], 0.0)
    nc.vector.tensor_tensor_scan(F2[:, 1:1 + NB * H], G[:, :NB * H],
                                 _sv(F2[:, 0:1], [(0, NB * H)]), 0.0, add, byp)
    for s in POOL_SIZES:
        jb = JBASE[s]
        hb = HBINS[s]
        for (o0, cnt, ds, de) in _uniform_runs(hb):
            hs, he = hb[o0]
            nc.vector.tensor_sub(
                _sv(P_out[:, B280[s] + o0 * s:], [(s, cnt), (1, s)]),
                _sv(F2[:, jb * H + he:], [(de, cnt), (H, s)]),
                _sv(F2[:, jb * H + hs:], [(ds, cnt), (H, s)]),
            )


def _emit_kpool(nc, D, kq, F1, G, F2):
    """K-side: kq [(third,kk)=96, TH*W=3072] bf16 -> D [96, 280] f32 raw
    third-local bin sums.  G layout [96, NB*TH] flat j*32+h."""
    add, byp = mybir.AluOpType.add, mybir.AluOpType.bypass
    n = TH * W
    nc.vector.memset(F1[:96, 0:1], 0.0)
    nc.vector.tensor_tensor_scan(F1[:96, 1:1 + n], kq[:, :],
                                 _sv(F1[:96, 0:1], [(0, n)]), 0.0, add, byp)
    jbase = 0
    for s in POOL_SIZES:
        sb = _pool_bins(W, s)
        for (i0, cnt, ds, de) in _uniform_runs(sb):
            ws, we = sb[i0]
            j0 = jbase + i0
            nc.vector.tensor_sub(
                _sv(G[:96, j0 * TH:], [(TH, cnt), (1, TH)]),
                _sv(F1[:96, we:], [(de, cnt), (W, TH)]),
                _sv(F1[:96, ws:], [(ds, cnt), (W, TH)]),
            )
        jbase += s
    nc.vector.memset(F2[:96, 0:1], 0.0)
    nc.vector.tensor_tensor_scan(F2[:96, 1:1 + NB * TH], G[:96, :NB * TH],
                                 _sv(F2[:96, 0:1], [(0, NB * TH)]), 0.0, add, byp)
    nc.vector.memset(D[:, :], 0.0)
    # third-local clipped H-bin diffs, per third partition group
    for t in range(3):
        for s in POOL_SIZES:
            jb = JBASE[s]
            # clipped (lhs, lhe) per output bin o
            cb = []
            for o, (hs, he) in enumerate(HBINS[s]):
                lhs = min(max(hs - TH * t, 0), TH)
                lhe = min(max(he - TH * t, 0), TH)
                cb.append((o, lhs, lhe))
            cb = [(o, a, b) for (o, a, b) in cb if b > a]
            # one instruction per clipped bin (multi-bin strided runs mis-lower)
            for (o0, a0, b0) in cb:
                nc.vector.tensor_sub(
                    _sv(D[32 * t:32 * t + 32, B280[s] + o0 * s:], [(1, s)]),
                    _sv(F2[32 * t:32 * t + 32, jb * TH + b0:], [(TH, s)]),
                    _sv(F2[32 * t:32 * t + 32, jb * TH + a0:], [(TH, s)]),
                )


def _fix_ldweights_waits(nc):
    """Move waits that gate weight data from InstMatmult to its InstLdweights
    (prevents stale-weight races after Tile's 2-byte matmul split)."""
    import copy
    for f in nc.m.functions:
        for bb in f.blocks:
            insts = bb.instructions
            new_list = []
            i = 0
            while i < len(insts):
                inst = insts[i]
                nxt = insts[i + 1] if i + 1 < len(insts) else None
                if (type(inst).__name__ == "InstLdweights" and nxt is not None
                        and type(nxt).__name__ == "InstMatmult"):
                    wl = list(inst.sync_info.on_wait) if inst.sync_info and inst.sync_info.on_wait else []
                    wm = list(nxt.sync_info.on_wait) if nxt.sync_info and nxt.sync_info.on_wait else []
                    waits = wl + wm
                    mm_upd = list(nxt.sync_info.on_update) if nxt.sync_info and nxt.sync_info.on_update else []
                    ld_upd = list(inst.sync_info.on_update) if inst.sync_info and inst.sync_info.on_update else []
                    if len(waits) > 1:
                        for k, w in enumerate(waits[:-1]):
                            pre = copy.deepcopy(inst)
                            pre.name = f"{inst.name}-ldw{k}"
                            pre.sync_info = bass_rust.SyncInfo(on_wait=[w], on_update=[])
                            new_list.append(pre)
                        inst.sync_info = bass_rust.SyncInfo(on_wait=[waits[-1]], on_update=ld_upd)
                        nxt.sync_info = bass_rust.SyncInfo(on_wait=[], on_update=mm_upd)
                    elif len(waits) == 1:
                        inst.sync_info = bass_rust.SyncInfo(on_wait=[waits[0]], on_update=ld_upd)
                        nxt.sync_info = bass_rust.SyncInfo(on_wait=[], on_update=mm_upd)
                    new_list.append(inst)
                    new_list.append(nxt)
                    i += 2
                    continue
                new_list.append(inst)
                i += 1
            bb.instructions[:] = new_list


def build_kernel(split_ctrl=True):
    nc = bacc.Bacc("TRN2", target_bir_lowering=False, debug=False)

    x1 = nc.dram_tensor("x1", [2, 128, HW], BF16, kind="ExternalInput")
    xx = nc.dram_tensor("xx", [2, 128, HW], BF16, kind="ExternalInput")
    wqT = nc.dram_tensor("wqT", [2, 128, C], F32, kind="ExternalInput")
    wkTb = nc.dram_tensor("wkTb", [2, 128, C8], BF16, kind="ExternalInput")
    bqb = nc.dram_tensor("bqb", [128, C], F32, kind="ExternalInput")
    bkb = nc.dram_tensor("bkb", [128, C8], F32, kind="ExternalInput")
    conz = nc.dram_tensor("conz", [6, 128, 96], BF16, kind="ExternalInput")
    ainv = nc.dram_tensor("ainv", [128, NP], F32, kind="ExternalInput")
    aiT3 = nc.dram_tensor("aiT3", [3, 128, C8], F32, kind="ExternalInput")
    identh = nc.dram_tensor("identh", [128, 128], F32, kind="ExternalInput")
    out = nc.dram_tensor("out", [2, 128, HW], BF16, kind="ExternalOutput")

    with tile.TileContext(nc) as tc:
        with (
            tc.tile_pool(name="consts", bufs=1) as cpool,
            tc.tile_pool(name="xpool", bufs=1) as xpool,
            tc.tile_pool(name="scratch", bufs=1) as spool,
            tc.tile_pool(name="ostage", bufs=4) as opool,
        ):
            # ---- constants ----
            wq_t = [cpool.tile([128, C], F32, tag=f"wq{i}", name=f"wq{i}") for i in range(2)]
            wk_t = [cpool.tile([128, C8], BF16, tag=f"wk{i}", name=f"wk{i}") for i in range(2)]
            bq_t = cpool.tile([128, C], F32, tag="bq", name="bq")
            bk_t = cpool.tile([128, C8], F32, tag="bk", name="bk")
            ai_t = cpool.tile([128, NP], F32, tag="ai", name="ai")
            ai3_t = [cpool.tile([128, C8], F32, tag=f"ai3{i}", name=f"ai3{i}") for i in range(3)]
            cz_t = [cpool.tile([128, 96], BF16, tag=f"cz{i}", name=f"cz{i}") for i in range(6)]
            id_t = cpool.tile([128, 128], F32, tag="idt", name="idt")
            for i in range(2):
                nc.sync.dma_start(wq_t[i][:], wqT.ap()[i])
                nc.sync.dma_start(wk_t[i][:], wkTb.ap()[i])
            nc.sync.dma_start(bq_t[:], bqb.ap())
            nc.sync.dma_start(bk_t[:], bkb.ap())
            nc.sync.dma_start(ai_t[:], ainv.ap())
            for i in range(3):
                nc.sync.dma_start(ai3_t[i][:], aiT3.ap()[i])
            for i in range(6):
                nc.sync.dma_start(cz_t[i][:], conz.ap()[i])
            nc.sync.dma_start(id_t[:], identh.ap())

            # ---- x_1 into zero-padded [128, 98, 98] tiles (conv rhs + q pooling) ----
            xpad = [xpool.tile([128, H + 2, W + 2], BF16, tag=f"xpad{i}", name=f"xpad{i}") for i in range(2)]
            for cc in range(2):
                nc.vector.memset(xpad[cc][:, 0:H + 2:H + 1, :], 0.0)
                nc.vector.memset(xpad[cc][:, 1:H + 1, 0:W + 2:W + 1], 0.0)
            x1v = [x1.ap()[cc].rearrange("p (h w) -> p h w", w=W) for cc in range(2)]
            for cc in range(2):
                for h0 in range(0, H, 24):
                    nc.sync.dma_start(
                        xpad[cc][:, 1 + h0:1 + h0 + 24, 1:1 + W],
                        x1v[cc][:, h0:h0 + 24, :],
                    )
            # ---- x (k-side input) staged whole ----
            xxs = [xpool.tile([128, HW], BF16, tag=f"xxs{i}", name=f"xxs{i}") for i in range(2)]
            for cc in range(2):
                for hh in range(2):
                    nc.sync.dma_start(xxs[cc][:, hh * (HW // 2):(hh + 1) * (HW // 2)],
                                      xx.ap()[cc][:, hh * (HW // 2):(hh + 1) * (HW // 2)])

            # ---- scratch ----
            F1 = spool.tile([128, STRIP * (W + 2) + 1], F32, tag="F1", name="F1")
            G = spool.tile([128, NB * H], F32, tag="G", name="G")
            F2 = spool.tile([128, NB * H + 1], F32, tag="F2", name="F2")
            Pq = [cpool.tile([128, NP], F32, tag=f"Pq{i}", name=f"Pq{i}") for i in range(2)]
            kq = spool.tile([96, TH * W], BF16, tag="kq", name="kq")
            Dk = spool.tile([96, NP], F32, tag="Dk", name="Dk")
            Dsh = [spool.tile([32, NP], F32, tag=f"Dsh{t}", name=f"Dsh{t}") for t in range(2)]
            zs = spool.tile([96, HW], BF16, tag="zs", name="zs")

            # ---- k = wk @ x, repacked into thirds via tile_position ----
            KCH = 512
            with tc.tile_pool(name="pk", bufs=2, space="PSUM") as pkp:
                for ch in range(HW // KCH):
                    t = ch // 6
                    off = (ch % 6) * KCH
                    kp = pkp.tile([128, KCH], F32, tag="kp", name="kp")
                    for cc in range(2):
                        nc.tensor.matmul(kp[32 * t:32 * t + 32, :], wk_t[cc][:],
                                         xxs[cc][:, ch * KCH:(ch + 1) * KCH],
                                         start=(cc == 0), stop=(cc == 1),
                                         tile_position=(0, 32 * t))
                    nc.scalar.copy(kq[32 * t:32 * t + 32, off:off + KCH],
                                   kp[32 * t:32 * t + 32, :])

            # ---- pooling: q side (2 chunks on x_1), k side (thirds) ----
            xpflat = [xpad[cc][:].rearrange("p h w -> p (h w)") for cc in range(2)]
            for cc in range(2):
                def src_x1(strip, _cc=cc):
                    return (xpflat[_cc][:, (W + 2):(1 + H) * (W + 2)], W + 2, 1)
                _emit_psp(nc, Pq[cc], src_x1, F1, G, F2)
            _emit_kpool(nc, Dk, kq, F1, G, F2)
            # matmul operands must sit at partition base 0: shift thirds 1,2 down
            for t in range(2):
                nc.sync.dma_start(Dsh[t][:, :], Dk[32 * (t + 1):32 * (t + 2), :])

            for cc in range(2):
                nc.vector.tensor_mul(Pq[cc][:], Pq[cc][:], ai_t[:])

            # ---- projections / affinity (affT96 = sigmoid(logits) replicated 3x) ----
            PCH = [(0, 128), (128, 128), (256, 24)]
            pqT = [cpool.tile([n, C], F32, tag=f"pqT{i}", name=f"pqT{i}") for i, (_, n) in enumerate(PCH)]
            pkR = [cpool.tile([n, 96], F32, tag=f"pkR{i}", name=f"pkR{i}") for i, (_, n) in enumerate(PCH)]
            mk1 = cpool.tile([128, C8], F32, tag="mk1", name="mk1")
            affT96 = cpool.tile([96, C], BF16, tag="affT96", name="affT96")

            with tc.tile_pool(name="psmall", bufs=2, space="PSUM") as pps:
                for i, (p0, n) in enumerate(PCH):
                    ps = pps.tile([n, C], F32, tag="ps", name="ps")
                    for cc in range(2):
                        nc.tensor.matmul(ps[:], Pq[cc][:, p0:p0 + n], wq_t[cc][:],
                                         start=(cc == 0), stop=(cc == 1))
                    nc.vector.tensor_add(pqT[i][:], ps[:], bq_t[:n, :])
                for i, (p0, n) in enumerate(PCH):
                    # pkT[p, kk] = sum_t Dk[(t,kk), p] via identity matmuls
                    ps2 = pps.tile([n, C8], F32, tag="ps2", name="ps2")
                    dsrc = [Dk, Dsh[0], Dsh[1]]
                    for t in range(3):
                        nc.tensor.matmul(ps2[:], dsrc[t][0:32, p0:p0 + n],
                                         id_t[0:32, 0:32],
                                         start=(t == 0), stop=(t == 2))
                    nc.vector.tensor_mul(mk1[:n, :], ps2[:], ai3_t[i][:n, :])
                    for g in range(3):
                        nc.vector.tensor_add(pkR[i][:, 32 * g:32 * g + 32],
                                             mk1[:n, :], bk_t[:n, :])
                pa = pps.tile([96, C], F32, tag="pa", name="pa")
                for i in range(3):
                    nc.tensor.matmul(pa[:], pkR[i][:], pqT[i][:],
                                     start=(i == 0), stop=(i == 2))
                nc.scalar.activation(affT96[:], pa[:], mybir.ActivationFunctionType.Sigmoid)

            # ---- zpart conv + dy-shifted copies + out matmuls ----
            nc.vector.memset(zs[0:32, 0:W], 0.0)
            nc.vector.memset(zs[64:96, (H - 1) * W:HW], 0.0)
            with (tc.tile_pool(name="pz", bufs=3, space="PSUM") as pzp,
                  tc.tile_pool(name="po", bufs=3, space="PSUM") as pop):
                for j in range(NCH):
                    r0 = j * ROWS
                    zp = pzp.tile([96, ROWS * W], F32, tag="zp", name="zp")
                    ti = 0
                    for dx in range(3):
                        for cinc in range(2):
                            rhs = xpad[cinc][:, 1 + r0:1 + r0 + ROWS, dx:dx + W]
                            nc.tensor.matmul(zp[:, :], cz_t[dx * 2 + cinc][:], rhs,
                                             start=(ti == 0), stop=(ti == 5))
                            ti += 1
                    # dy=1 (center): aligned
                    nc.scalar.copy(zs[32:64, r0 * W:(r0 + ROWS) * W], zp[32:64, :])
                    # dy=0: shift down one row (zs[h] = zp[h-1])
                    if r0 + ROWS < H:
                        nc.scalar.copy(zs[0:32, (r0 + 1) * W:(r0 + ROWS + 1) * W], zp[0:32, :])
                    else:
                        nc.scalar.copy(zs[0:32, (r0 + 1) * W:HW], zp[0:32, :(ROWS - 1) * W])
                    # dy=2: shift up one row (zs[h] = zp[h+1])
                    if r0 == 0:
                        nc.scalar.copy(zs[64:96, 0:(ROWS - 1) * W], zp[64:96, W:])
                    else:
                        nc.scalar.copy(zs[64:96, (r0 - 1) * W:(r0 + ROWS - 1) * W], zp[64:96, :])
                for j in range(NCH):
                    r0 = j * ROWS
                    for coutc in range(2):
                        op = pop.tile([128, ROWS * W], F32, tag="op", name="op")
                        nc.tensor.matmul(op[:], affT96[:, coutc * 128:(coutc + 1) * 128],
                                         zs[:, r0 * W:(r0 + ROWS) * W],
                                         start=True, stop=True)
                        ot = opool.tile([128, ROWS * W], BF16, tag="ot", name="ot")
                        nc.scalar.copy(ot[:], op[:])
                        nc.sync.dma_start(out.ap()[coutc][:, r0 * W:(r0 + ROWS) * W], ot[:])

    if split_ctrl:
        nc.compile()
        _fix_ldweights_waits(nc)
    return nc


_NC_CACHE = {}


def _get_nc():
    if "nc" not in _NC_CACHE:
        _NC_CACHE["nc"] = build_kernel()
    return _NC_CACHE["nc"]


def _conv_cast(x):
    import ml_dtypes
    return np.ascontiguousarray(x, np.float32).astype(ml_dtypes.bfloat16)


def kernel(x_1, x, wq, bq, wk, bk, con):
    import ml_dtypes
    x_1 = _conv_cast(x_1)
    x = _conv_cast(x)
    con = np.asarray(con, np.float32)
    wq = np.asarray(wq, np.float32)
    bq = np.asarray(bq, np.float32)
    wk = np.asarray(wk, np.float32)
    bk = np.asarray(bk, np.float32)

    wqT_h = np.ascontiguousarray(wq.T).reshape(2, 128, C)
    wkTb_h = np.ascontiguousarray(wk.T).reshape(2, 128, C8).astype(ml_dtypes.bfloat16)
    bqb_h = np.tile(bq, (128, 1))
    bkb_h = np.tile(bk, (128, 1))
    # conz[dx*2+cinc, cin, dy*32+kk] = con[kk, cinc*128+cin, dy, dx]
    conz_h = np.ascontiguousarray(
        con.transpose(3, 1, 2, 0)          # [dx, cin256, dy, kk]
        .reshape(3, 2, 128, 3 * C8)
        .transpose(0, 1, 2, 3)
        .reshape(6, 128, 96)
    ).astype(ml_dtypes.bfloat16)
    ai = _area_inv()
    ainv_h = np.tile(ai, (128, 1))
    aiT3_h = np.zeros((3, 128, C8), np.float32)
    for i, (p0, n) in enumerate([(0, 128), (128, 128), (256, 24)]):
        aiT3_h[i, :n, :] = ai[p0:p0 + n, None]
    ident_h = np.eye(128, dtype=np.float32)

    in_maps = []
    for b in range(B):
        in_maps.append({
            "x1": x_1[b].reshape(2, 128, HW),
            "xx": x[b].reshape(2, 128, HW),
            "wqT": wqT_h, "wkTb": wkTb_h, "bqb": bqb_h, "bkb": bkb_h,
            "conz": conz_h, "ainv": ainv_h, "aiT3": aiT3_h, "identh": ident_h,
        })
    global _last_in_maps
    _last_in_maps = in_maps
    nc = _get_nc()
    res = run_bass_kernel_spmd(nc, in_maps, list(range(B)))
    return np.stack([res.results[b]["out"].astype(np.float32).reshape(C, H, W)
                     for b in range(B)])


# revision 10
# speedup vs baseline: 2.0911x; 1.2422x over previous
"""Trainium2 Bass kernel for nn_ASSC_66657892434080.

Reference computation (per batch sample b, data-parallel over 8 cores):
    q = wq @ x_1[b] + bq ; k = wk @ x[b] + bk          (1x1 convs)
    proj_query = PSP(q) [256,280] ; proj_key = PSP(k) [32,280]
    aff = sigmoid(proj_query @ proj_key^T)             [256,32]
    agg_w = (aff @ con.reshape(32, 256*9))             per-sample 3x3 weights
    out[b] = conv3x3(x_1[b], agg_w)                    (grouped conv, groups=B)

Restructuring (validated vs reference in numpy to ~5e-7):
  * conv(x, aff@con) == aff @ conv(x, con): the big per-sample conv becomes a
    32-channel conv z = conv3x3(x_1, con) plus a small matmul out = aff @ z.
    PE columns drop ~4.7x vs synthesizing [256,256,3,3] weights.
  * z computed as zpart[(dy,k), h, w] (dy packed into PE output columns, M=96)
    from 6 accumulating matmuls (3 dx shifts x 2 cin chunks); the dy row-shift
    is applied during the PSUM->SBUF (bf16) copy; out = affrep(96) @ zs.
  * PSP is linear: PSP(wq@x+bq) = wq@PSP(x)+bq (q side pools x_1 directly).
  * K side projects FIRST (k = wk@x on PE, 32 ch), k is repacked into thirds
    [(third,kk)=96 partitions, 32h x 96w] via matmul tile_position, so the
    pooling scans are 3x shorter; third-local bin sums are combined by
    accumulating identity-matmul transposes into pkT [p,32] tiles.
  * Output DMA'd in bf16 and upcast on host (rel-err budget 2e-2).
"""

import numpy as np
import concourse.bass as bass
import concourse.bacc as bacc
import concourse.tile as tile
import concourse.mybir as mybir
import bass_rust
from concourse.bass_utils import run_bass_kernel_spmd

B, C, H, W = 8, 256, 96, 96
C8 = 32
HW = H * W                      # 9216
POOL_SIZES = (1, 3, 5, 7, 14)   # -> 30 1-D bins, 280 2-D positions
NB = sum(POOL_SIZES)            # 30
NP = sum(s * s for s in POOL_SIZES)  # 280
STRIP = 96                      # q-side pooling strip rows (full chunk)
TH = 32                         # k-side rows per third
ROWS = 4                        # conv rows per PSUM chunk
NCH = H // ROWS                 # 24 chunks
F32 = mybir.dt.float32
BF16 = mybir.dt.bfloat16


def _pool_bins(n, s):
    return [((i * n) // s, -((-(i + 1) * n) // s)) for i in range(s)]


HBINS = {s: _pool_bins(H, s) for s in POOL_SIZES}
JBASE = {}
B280 = {}
_j = _p = 0
for _s in POOL_SIZES:
    JBASE[_s] = _j
    B280[_s] = _p
    _j += _s
    _p += _s * _s


def _area_inv():
    ai = np.zeros(NP, np.float32)
    for s in POOL_SIZES:
        hb, wb = _pool_bins(H, s), _pool_bins(W, s)
        for o, (hs, he) in enumerate(hb):
            for p, (ws, we) in enumerate(wb):
                ai[B280[s] + o * s + p] = 1.0 / ((he - hs) * (we - ws))
    return ai


def _split_multiwait_ctrl(nc, default_limit=1):
    """walrus in this container rejects instructions carrying more than one
    sem wait; move extras onto preceding same-engine drains.  NEVER split PE
    instructions (reorder window pulls LDWEIGHTS ahead)."""
    for f in nc.m.functions:
        for bb in f.blocks:
            new_list = []
            for inst in bb.instructions:
                si = inst.sync_info
                waits = list(si.on_wait) if si and si.on_wait else []
                mw = default_limit
                if getattr(inst, "engine", None) == mybir.EngineType.PE:
                    mw = 99
                if len(waits) > mw:
                    for k, w in enumerate(waits[:-mw]):
                        pre = mybir.InstDrain(name=f"{inst.name}-w{k}", ins=[], outs=[])
                        pre.engine = inst.engine
                        pre.sync_info = bass_rust.SyncInfo(on_wait=[w], on_update=[])
                        new_list.append(pre)
                    inst.sync_info = bass_rust.SyncInfo(
                        on_wait=waits[-mw:],
                        on_update=list(si.on_update) if si.on_update else [],
                    )
                new_list.append(inst)
            bb.instructions[:] = new_list


def _sv(ap2d, dims):
    """Strided view: keep partition dim, replace free dims with (step, count)."""
    import dataclasses
    return dataclasses.replace(ap2d, ap=[list(ap2d.ap[0])] + [[s, c] for s, c in dims])


def _uniform_runs(bins):
    """Group consecutive bins into runs with constant boundary strides."""
    runs = []
    i = 0
    n = len(bins)
    while i < n:
        if i == n - 1:
            runs.append((i, 1, 0, 0))
            i += 1
            continue
        ds = bins[i + 1][0] - bins[i][0]
        de = bins[i + 1][1] - bins[i][1]
        j = i + 1
        while (j + 1 < n and bins[j + 1][0] - bins[j][0] == ds
               and bins[j + 1][1] - bins[j][1] == de):
            j += 1
        runs.append((i, j - i + 1, ds, de))
        i = j + 1
    return runs


def _emit_psp(nc, P_out, src2d, F1, G, F2, nstrip=1):
    """Q-side: pool one 128-channel chunk into P_out [128, 280] (raw SUMS).
    G layout: [128, NB*H] flat j*96+h."""
    add, byp = mybir.AluOpType.add, mybir.AluOpType.bypass
    rows = H // nstrip
    for strip in range(nstrip):
        h0 = strip * rows
        src, rowlen, coff = src2d(strip)
        n = rows * rowlen
        nc.vector.memset(F1[:, 0:1], 0.0)
        nc.vector.tensor_tensor_scan(F1[:, 1:1 + n], src, _sv(F1[:, 0:1], [(0, n)]),
                                     0.0, add, byp)
        jbase = 0
        for s in POOL_SIZES:
            sb = _pool_bins(W, s)
            for (i0, cnt, ds, de) in _uniform_runs(sb):
                ws, we = sb[i0]
                j0 = jbase + i0
                nc.vector.tensor_sub(
                    _sv(G[:, j0 * H + h0:], [(H, cnt), (1, rows)]),
                    _sv(F1[:, coff + we:], [(de, cnt), (rowlen, rows)]),
                    _sv(F1[:, coff + ws:], [(ds, cnt), (rowlen, rows)]),
                )
            jbase += s
    nc.vector.memset(F2[:, 0:1], 0.0)
    nc.vector.tensor_tensor_scan(F2[:, 1:1 + NB * H], G[:, :NB * H],
                                 _sv(F2[:, 0:1], [(0, NB * H)]), 0.0, add, byp)
    for s in POOL_SIZES:
        jb = JBASE[s]
        hb = HBINS[s]
        for (o0, cnt, ds, de) in _uniform_runs(hb):
            hs, he = hb[o0]
            nc.vector.tensor_sub(
                _sv(P_out[:, B280[s] + o0 * s:], [(s, cnt), (1, s)]),
                _sv(F2[:, jb * H + he:], [(de, cnt), (H, s)]),
                _sv(F2[:, jb * H + hs:], [(ds, cnt), (H, s)]),
            )


def _emit_kpool(nc, D, kq, F1, G, F2):
    """K-side: kq [(third,kk)=96, TH*W=3072] bf16 -> D [96, 280] f32 raw
    third-local bin sums.  G layout [96, NB*TH] flat j*32+h."""
    add, byp = mybir.AluOpType.add, mybir.AluOpType.bypass
    n = TH * W
    nc.vector.memset(F1[:96, 0:1], 0.0)
    nc.vector.tensor_tensor_scan(F1[:96, 1:1 + n], kq[:, :],
                                 _sv(F1[:96, 0:1], [(0, n)]), 0.0, add, byp)
    jbase = 0
    for s in POOL_SIZES:
        sb = _pool_bins(W, s)
        for (i0, cnt, ds, de) in _uniform_runs(sb):
            ws, we = sb[i0]
            j0 = jbase + i0
            nc.vector.tensor_sub(
                _sv(G[:96, j0 * TH:], [(TH, cnt), (1, TH)]),
                _sv(F1[:96, we:], [(de, cnt), (W, TH)]),
                _sv(F1[:96, ws:], [(ds, cnt), (W, TH)]),
            )
        jbase += s
    nc.vector.memset(F2[:96, 0:1], 0.0)
    nc.vector.tensor_tensor_scan(F2[:96, 1:1 + NB * TH], G[:96, :NB * TH],
                                 _sv(F2[:96, 0:1], [(0, NB * TH)]), 0.0, add, byp)
    nc.vector.memset(D[:, :], 0.0)
    # third-local clipped H-bin diffs, per third partition group
    for t in range(3):
        for s in POOL_SIZES:
            jb = JBASE[s]
            # clipped (lhs, lhe) per output bin o
            cb = []
            for o, (hs, he) in enumerate(HBINS[s]):
                lhs = min(max(hs - TH * t, 0), TH)
                lhe = min(max(he - TH * t, 0), TH)
                cb.append((o, lhs, lhe))
            cb = [(o, a, b) for (o, a, b) in cb if b > a]
            # one instruction per clipped bin (multi-bin strided runs mis-lower)
            for (o0, a0, b0) in cb:
                nc.vector.tensor_sub(
                    _sv(D[32 * t:32 * t + 32, B280[s] + o0 * s:], [(1, s)]),
                    _sv(F2[32 * t:32 * t + 32, jb * TH + b0:], [(TH, s)]),
                    _sv(F2[32 * t:32 * t + 32, jb * TH + a0:], [(TH, s)]),
                )


def _fix_ldweights_waits(nc):
    """Move waits that gate weight data from InstMatmult to its InstLdweights
    (prevents stale-weight races after Tile's 2-byte matmul split)."""
    import copy
    for f in nc.m.functions:
        for bb in f.blocks:
            insts = bb.instructions
            new_list = []
            i = 0
            while i < len(insts):
                inst = insts[i]
                nxt = insts[i + 1] if i + 1 < len(insts) else None
                if (type(inst).__name__ == "InstLdweights" and nxt is not None
                        and type(nxt).__name__ == "InstMatmult"):
                    wl = list(inst.sync_info.on_wait) if inst.sync_info and inst.sync_info.on_wait else []
                    wm = list(nxt.sync_info.on_wait) if nxt.sync_info and nxt.sync_info.on_wait else []
                    waits = wl + wm
                    mm_upd = list(nxt.sync_info.on_update) if nxt.sync_info and nxt.sync_info.on_update else []
                    ld_upd = list(inst.sync_info.on_update) if inst.sync_info and inst.sync_info.on_update else []
                    if len(waits) > 1:
                        for k, w in enumerate(waits[:-1]):
                            pre = copy.deepcopy(inst)
                            pre.name = f"{inst.name}-ldw{k}"
                            pre.sync_info = bass_rust.SyncInfo(on_wait=[w], on_update=[])
                            new_list.append(pre)
                        inst.sync_info = bass_rust.SyncInfo(on_wait=[waits[-1]], on_update=ld_upd)
                        nxt.sync_info = bass_rust.SyncInfo(on_wait=[], on_update=mm_upd)
                    elif len(waits) == 1:
                        inst.sync_info = bass_rust.SyncInfo(on_wait=[waits[0]], on_update=ld_upd)
                        nxt.sync_info = bass_rust.SyncInfo(on_wait=[], on_update=mm_upd)
                    new_list.append(inst)
                    new_list.append(nxt)
                    i += 2
                    continue
                new_list.append(inst)
                i += 1
            bb.instructions[:] = new_list


def build_kernel(split_ctrl=True):
    nc = bacc.Bacc("TRN2", target_bir_lowering=False, debug=False)

    x1 = nc.dram_tensor("x1", [2, 128, HW], BF16, kind="ExternalInput")
    xx = nc.dram_tensor("xx", [2, 128, HW], BF16, kind="ExternalInput")
    wqT = nc.dram_tensor("wqT", [2, 128, C], F32, kind="ExternalInput")
    wkTb = nc.dram_tensor("wkTb", [2, 128, C8], BF16, kind="ExternalInput")
    bqb = nc.dram_tensor("bqb", [128, C], F32, kind="ExternalInput")
    bkb = nc.dram_tensor("bkb", [128, C8], F32, kind="ExternalInput")
    conz = nc.dram_tensor("conz", [6, 128, 96], BF16, kind="ExternalInput")
    ainv = nc.dram_tensor("ainv", [128, NP], F32, kind="ExternalInput")
    aiT3 = nc.dram_tensor("aiT3", [3, 128, C8], F32, kind="ExternalInput")
    identh = nc.dram_tensor("identh", [128, 128], F32, kind="ExternalInput")
    out = nc.dram_tensor("out", [2, 128, HW], BF16, kind="ExternalOutput")

    with tile.TileContext(nc) as tc:
        with (
            tc.tile_pool(name="consts", bufs=1) as cpool,
            tc.tile_pool(name="xpool", bufs=1) as xpool,
            tc.tile_pool(name="scratch", bufs=1) as spool,
            tc.tile_pool(name="ostage", bufs=4) as opool,
        ):
            # ---- constants ----
            wq_t = [cpool.tile([128, C], F32, tag=f"wq{i}", name=f"wq{i}") for i in range(2)]
            wk_t = [cpool.tile([128, C8], BF16, tag=f"wk{i}", name=f"wk{i}") for i in range(2)]
            bq_t = cpool.tile([128, C], F32, tag="bq", name="bq")
            bk_t = cpool.tile([128, C8], F32, tag="bk", name="bk")
            ai_t = cpool.tile([128, NP], F32, tag="ai", name="ai")
            ai3_t = [cpool.tile([128, C8], F32, tag=f"ai3{i}", name=f"ai3{i}") for i in range(3)]
            cz_t = [cpool.tile([128, 96], BF16, tag=f"cz{i}", name=f"cz{i}") for i in range(6)]
            id_t = cpool.tile([128, 128], F32, tag="idt", name="idt")
            for i in range(2):
                nc.sync.dma_start(wq_t[i][:], wqT.ap()[i])
                nc.sync.dma_start(wk_t[i][:], wkTb.ap()[i])
            nc.sync.dma_start(bq_t[:], bqb.ap())
            nc.sync.dma_start(bk_t[:], bkb.ap())
            nc.sync.dma_start(ai_t[:], ainv.ap())
            for i in range(3):
                nc.sync.dma_start(ai3_t[i][:], aiT3.ap()[i])
            for i in range(6):
                nc.sync.dma_start(cz_t[i][:], conz.ap()[i])
            nc.sync.dma_start(id_t[:], identh.ap())

            # ---- x_1 into zero-padded [128, 98, 98] tiles (conv rhs + q pooling) ----
            xpad = [xpool.tile([128, H + 2, W + 2], BF16, tag=f"xpad{i}", name=f"xpad{i}") for i in range(2)]
            for cc in range(2):
                nc.vector.memset(xpad[cc][:, 0:H + 2:H + 1, :], 0.0)
                nc.vector.memset(xpad[cc][:, 1:H + 1, 0:W + 2:W + 1], 0.0)
            x1v = [x1.ap()[cc].rearrange("p (h w) -> p h w", w=W) for cc in range(2)]
            for cc in range(2):
                for h0 in range(0, H, 24):
                    nc.sync.dma_start(
                        xpad[cc][:, 1 + h0:1 + h0 + 24, 1:1 + W],
                        x1v[cc][:, h0:h0 + 24, :],
                    )
            # ---- x (k-side input) staged whole ----
            xxs = [xpool.tile([128, HW], BF16, tag=f"xxs{i}", name=f"xxs{i}") for i in range(2)]
            for cc in range(2):
                for hh in range(2):
                    nc.sync.dma_start(xxs[cc][:, hh * (HW // 2):(hh + 1) * (HW // 2)],
                                      xx.ap()[cc][:, hh * (HW // 2):(hh + 1) * (HW // 2)])

            # ---- scratch ----
            F1 = spool.tile([128, STRIP * (W + 2) + 1], F32, tag="F1", name="F1")
            G = spool.tile([128, NB * H], F32, tag="G", name="G")
            F2 = spool.tile([128, NB * H + 1], F32, tag="F2", name="F2")
            Pq = [cpool.tile([128, NP], F32, tag=f"Pq{i}", name=f"Pq{i}") for i in range(2)]
            kq = spool.tile([96, TH * W], BF16, tag="kq", name="kq")
            Dk = spool.tile([96, NP], F32, tag="Dk", name="Dk")
            Dsh = [spool.tile([32, NP], F32, tag=f"Dsh{t}", name=f"Dsh{t}") for t in range(2)]

            # ---- k = wk @ x, repacked into thirds via tile_position ----
            KCH = 512
            with tc.tile_pool(name="pk", bufs=2, space="PSUM") as pkp:
                for ch in range(HW // KCH):
                    t = ch // 6
                    off = (ch % 6) * KCH
                    kp = pkp.tile([128, KCH], F32, tag="kp", name="kp")
                    for cc in range(2):
                        nc.tensor.matmul(kp[32 * t:32 * t + 32, :], wk_t[cc][:],
                                         xxs[cc][:, ch * KCH:(ch + 1) * KCH],
                                         start=(cc == 0), stop=(cc == 1),
                                         tile_position=(0, 32 * t))
                    nc.scalar.copy(kq[32 * t:32 * t + 32, off:off + KCH],
                                   kp[32 * t:32 * t + 32, :])

            # ---- zpart conv (independent of pooling: overlaps it on PE) ----
            zs = spool.tile([96, HW], BF16, tag="zs", name="zs")
            nc.vector.memset(zs[0:32, 0:W], 0.0)
            nc.vector.memset(zs[64:96, (H - 1) * W:HW], 0.0)
            with tc.tile_pool(name="pz", bufs=3, space="PSUM") as pzp:
                for j in range(NCH):
                    r0 = j * ROWS
                    zp = pzp.tile([96, ROWS * W], F32, tag="zp", name="zp")
                    ti = 0
                    for dx in range(3):
                        for cinc in range(2):
                            rhs = xpad[cinc][:, 1 + r0:1 + r0 + ROWS, dx:dx + W]
                            nc.tensor.matmul(zp[:, :], cz_t[dx * 2 + cinc][:], rhs,
                                             start=(ti == 0), stop=(ti == 5))
                            ti += 1
                    nc.scalar.copy(zs[32:64, r0 * W:(r0 + ROWS) * W], zp[32:64, :])
                    if r0 + ROWS < H:
                        nc.scalar.copy(zs[0:32, (r0 + 1) * W:(r0 + ROWS + 1) * W], zp[0:32, :])
                    else:
                        nc.scalar.copy(zs[0:32, (r0 + 1) * W:HW], zp[0:32, :(ROWS - 1) * W])
                    if r0 == 0:
                        nc.scalar.copy(zs[64:96, 0:(ROWS - 1) * W], zp[64:96, W:])
                    else:
                        nc.scalar.copy(zs[64:96, (r0 - 1) * W:(r0 + ROWS - 1) * W], zp[64:96, :])

            # ---- pooling: q side (2 chunks on x_1), k side (thirds) ----
            xpflat = [xpad[cc][:].rearrange("p h w -> p (h w)") for cc in range(2)]
            for cc in range(2):
                def src_x1(strip, _cc=cc):
                    return (xpflat[_cc][:, (W + 2):(1 + H) * (W + 2)], W + 2, 1)
                _emit_psp(nc, Pq[cc], src_x1, F1, G, F2)
            _emit_kpool(nc, Dk, kq, F1, G, F2)
            # matmul operands must sit at partition base 0: shift thirds 1,2 down
            for t in range(2):
                nc.sync.dma_start(Dsh[t][:, :], Dk[32 * (t + 1):32 * (t + 2), :])

            for cc in range(2):
                nc.vector.tensor_mul(Pq[cc][:], Pq[cc][:], ai_t[:])

            # ---- projections / affinity (affT96 = sigmoid(logits) replicated 3x) ----
            PCH = [(0, 128), (128, 128), (256, 24)]
            pqT = [cpool.tile([n, C], F32, tag=f"pqT{i}", name=f"pqT{i}") for i, (_, n) in enumerate(PCH)]
            pkR = [cpool.tile([n, 96], F32, tag=f"pkR{i}", name=f"pkR{i}") for i, (_, n) in enumerate(PCH)]
            mk1 = cpool.tile([128, C8], F32, tag="mk1", name="mk1")
            affT96 = cpool.tile([96, C], BF16, tag="affT96", name="affT96")

            with tc.tile_pool(name="psmall", bufs=2, space="PSUM") as pps:
                for i, (p0, n) in enumerate(PCH):
                    ps = pps.tile([n, C], F32, tag="ps", name="ps")
                    for cc in range(2):
                        nc.tensor.matmul(ps[:], Pq[cc][:, p0:p0 + n], wq_t[cc][:],
                                         start=(cc == 0), stop=(cc == 1))
                    nc.vector.tensor_add(pqT[i][:], ps[:], bq_t[:n, :])
                for i, (p0, n) in enumerate(PCH):
                    # pkT[p, kk] = sum_t Dk[(t,kk), p] via identity matmuls
                    ps2 = pps.tile([n, C8], F32, tag="ps2", name="ps2")
                    dsrc = [Dk, Dsh[0], Dsh[1]]
                    for t in range(3):
                        nc.tensor.matmul(ps2[:], dsrc[t][0:32, p0:p0 + n],
                                         id_t[0:32, 0:32],
                                         start=(t == 0), stop=(t == 2))
                    nc.vector.tensor_mul(mk1[:n, :], ps2[:], ai3_t[i][:n, :])
                    for g in range(3):
                        nc.vector.tensor_add(pkR[i][:, 32 * g:32 * g + 32],
                                             mk1[:n, :], bk_t[:n, :])
                pa = pps.tile([96, C], F32, tag="pa", name="pa")
                for i in range(3):
                    nc.tensor.matmul(pa[:], pkR[i][:], pqT[i][:],
                                     start=(i == 0), stop=(i == 2))
                nc.scalar.activation(affT96[:], pa[:], mybir.ActivationFunctionType.Sigmoid)

            # ---- out matmuls ----
            with tc.tile_pool(name="po", bufs=3, space="PSUM") as pop:
                for j in range(NCH):
                    r0 = j * ROWS
                    for coutc in range(2):
                        op = pop.tile([128, ROWS * W], F32, tag="op", name="op")
                        nc.tensor.matmul(op[:], affT96[:, coutc * 128:(coutc + 1) * 128],
                                         zs[:, r0 * W:(r0 + ROWS) * W],
                                         start=True, stop=True)
                        ot = opool.tile([128, ROWS * W], BF16, tag="ot", name="ot")
                        nc.scalar.copy(ot[:], op[:])
                        nc.sync.dma_start(out.ap()[coutc][:, r0 * W:(r0 + ROWS) * W], ot[:])

    if split_ctrl:
        nc.compile()
        _fix_ldweights_waits(nc)
    return nc


_NC_CACHE = {}


def _get_nc():
    if "nc" not in _NC_CACHE:
        _NC_CACHE["nc"] = build_kernel()
    return _NC_CACHE["nc"]


def _conv_cast(x):
    import ml_dtypes
    return np.ascontiguousarray(x, np.float32).astype(ml_dtypes.bfloat16)


def kernel(x_1, x, wq, bq, wk, bk, con):
    import ml_dtypes
    x_1 = _conv_cast(x_1)
    x = _conv_cast(x)
    con = np.asarray(con, np.float32)
    wq = np.asarray(wq, np.float32)
    bq = np.asarray(bq, np.float32)
    wk = np.asarray(wk, np.float32)
    bk = np.asarray(bk, np.float32)

    wqT_h = np.ascontiguousarray(wq.T).reshape(2, 128, C)
    wkTb_h = np.ascontiguousarray(wk.T).reshape(2, 128, C8).astype(ml_dtypes.bfloat16)
    bqb_h = np.tile(bq, (128, 1))
    bkb_h = np.tile(bk, (128, 1))
    # conz[dx*2+cinc, cin, dy*32+kk] = con[kk, cinc*128+cin, dy, dx]
    conz_h = np.ascontiguousarray(
        con.transpose(3, 1, 2, 0)          # [dx, cin256, dy, kk]
        .reshape(3, 2, 128, 3 * C8)
        .transpose(0, 1, 2, 3)
        .reshape(6, 128, 96)
    ).astype(ml_dtypes.bfloat16)
    ai = _area_inv()
    ainv_h = np.tile(ai, (128, 1))
    aiT3_h = np.zeros((3, 128, C8), np.float32)
    for i, (p0, n) in enumerate([(0, 128), (128, 128), (256, 24)]):
        aiT3_h[i, :n, :] = ai[p0:p0 + n, None]
    ident_h = np.eye(128, dtype=np.float32)

    in_maps = []
    for b in range(B):
        in_maps.append({
            "x1": x_1[b].reshape(2, 128, HW),
            "xx": x[b].reshape(2, 128, HW),
            "wqT": wqT_h, "wkTb": wkTb_h, "bqb": bqb_h, "bkb": bkb_h,
            "conz": conz_h, "ainv": ainv_h, "aiT3": aiT3_h, "identh": ident_h,
        })
    global _last_in_maps
    _last_in_maps = in_maps
    nc = _get_nc()
    res = run_bass_kernel_spmd(nc, in_maps, list(range(B)))
    return np.stack([res.results[b]["out"].astype(np.float32).reshape(C, H, W)
                     for b in range(B)])


# revision 12
# speedup vs baseline: 2.3000x; 1.0999x over previous
"""Trainium2 Bass kernel for nn_ASSC_66657892434080.

Reference computation (per batch sample b, data-parallel over 8 cores):
    q = wq @ x_1[b] + bq ; k = wk @ x[b] + bk          (1x1 convs)
    proj_query = PSP(q) [256,280] ; proj_key = PSP(k) [32,280]
    aff = sigmoid(proj_query @ proj_key^T)             [256,32]
    agg_w = (aff @ con.reshape(32, 256*9))             per-sample 3x3 weights
    out[b] = conv3x3(x_1[b], agg_w)                    (grouped conv, groups=B)

Restructuring (validated vs reference in numpy to ~5e-7):
  * conv(x, aff@con) == aff @ conv(x, con): the big per-sample conv becomes a
    32-channel conv z = conv3x3(x_1, con) plus a small matmul out = aff @ z.
    PE columns drop ~4.7x vs synthesizing [256,256,3,3] weights.
  * z computed as zpart[(dy,k), h, w] (dy packed into PE output columns, M=96)
    from 6 accumulating matmuls (3 dx shifts x 2 cin chunks); the dy row-shift
    is applied during the PSUM->SBUF (bf16) copy; out = affrep(96) @ zs.
  * PSP is linear: PSP(wq@x+bq) = wq@PSP(x)+bq (q side pools x_1 directly).
  * K side projects FIRST (k = wk@x on PE, 32 ch), k is repacked into thirds
    [(third,kk)=96 partitions, 32h x 96w] via matmul tile_position, so the
    pooling scans are 3x shorter; third-local bin sums are combined by
    accumulating identity-matmul transposes into pkT [p,32] tiles.
  * Output DMA'd in bf16 and upcast on host (rel-err budget 2e-2).
"""

import numpy as np
import concourse.bass as bass
import concourse.bacc as bacc
import concourse.tile as tile
import concourse.mybir as mybir
import bass_rust
from concourse.bass_utils import run_bass_kernel_spmd

B, C, H, W = 8, 256, 96, 96
C8 = 32
HW = H * W                      # 9216
POOL_SIZES = (1, 3, 5, 7, 14)   # -> 30 1-D bins, 280 2-D positions
NB = sum(POOL_SIZES)            # 30
NP = sum(s * s for s in POOL_SIZES)  # 280
STRIP = 96                      # q-side pooling strip rows (full chunk)
TH = 32                         # k-side rows per third
ROWS = 4                        # conv rows per PSUM chunk
NCH = H // ROWS                 # 24 chunks
F32 = mybir.dt.float32
BF16 = mybir.dt.bfloat16


def _pool_bins(n, s):
    return [((i * n) // s, -((-(i + 1) * n) // s)) for i in range(s)]


HBINS = {s: _pool_bins(H, s) for s in POOL_SIZES}
JBASE = {}
B280 = {}
_j = _p = 0
for _s in POOL_SIZES:
    JBASE[_s] = _j
    B280[_s] = _p
    _j += _s
    _p += _s * _s


def _area_inv():
    ai = np.zeros(NP, np.float32)
    for s in POOL_SIZES:
        hb, wb = _pool_bins(H, s), _pool_bins(W, s)
        for o, (hs, he) in enumerate(hb):
            for p, (ws, we) in enumerate(wb):
                ai[B280[s] + o * s + p] = 1.0 / ((he - hs) * (we - ws))
    return ai


def _split_multiwait_ctrl(nc, default_limit=1):
    """walrus in this container rejects instructions carrying more than one
    sem wait; move extras onto preceding same-engine drains.  NEVER split PE
    instructions (reorder window pulls LDWEIGHTS ahead)."""
    for f in nc.m.functions:
        for bb in f.blocks:
            new_list = []
            for inst in bb.instructions:
                si = inst.sync_info
                waits = list(si.on_wait) if si and si.on_wait else []
                mw = default_limit
                if getattr(inst, "engine", None) == mybir.EngineType.PE:
                    mw = 99
                if len(waits) > mw:
                    for k, w in enumerate(waits[:-mw]):
                        pre = mybir.InstDrain(name=f"{inst.name}-w{k}", ins=[], outs=[])
                        pre.engine = inst.engine
                        pre.sync_info = bass_rust.SyncInfo(on_wait=[w], on_update=[])
                        new_list.append(pre)
                    inst.sync_info = bass_rust.SyncInfo(
                        on_wait=waits[-mw:],
                        on_update=list(si.on_update) if si.on_update else [],
                    )
                new_list.append(inst)
            bb.instructions[:] = new_list


def _sv(ap2d, dims):
    """Strided view: keep partition dim, replace free dims with (step, count)."""
    import dataclasses
    return dataclasses.replace(ap2d, ap=[list(ap2d.ap[0])] + [[s, c] for s, c in dims])


def _uniform_runs(bins):
    """Group consecutive bins into runs with constant boundary strides."""
    runs = []
    i = 0
    n = len(bins)
    while i < n:
        if i == n - 1:
            runs.append((i, 1, 0, 0))
            i += 1
            continue
        ds = bins[i + 1][0] - bins[i][0]
        de = bins[i + 1][1] - bins[i][1]
        j = i + 1
        while (j + 1 < n and bins[j + 1][0] - bins[j][0] == ds
               and bins[j + 1][1] - bins[j][1] == de):
            j += 1
        runs.append((i, j - i + 1, ds, de))
        i = j + 1
    return runs


def _emit_psp(nc, P_out, src2d, F1, G, F2, nstrip=1):
    """Q-side: pool one 128-channel chunk into P_out [128, 280] (raw SUMS).
    G layout: [128, NB*H] flat j*96+h."""
    add, byp = mybir.AluOpType.add, mybir.AluOpType.bypass
    rows = H // nstrip
    for strip in range(nstrip):
        h0 = strip * rows
        src, rowlen, coff = src2d(strip)
        n = rows * rowlen
        nc.vector.memset(F1[:, 0:1], 0.0)
        nc.vector.tensor_tensor_scan(F1[:, 1:1 + n], src, _sv(F1[:, 0:1], [(0, n)]),
                                     0.0, add, byp)
        jbase = 0
        for s in POOL_SIZES:
            sb = _pool_bins(W, s)
            for (i0, cnt, ds, de) in _uniform_runs(sb):
                ws, we = sb[i0]
                j0 = jbase + i0
                nc.vector.tensor_sub(
                    _sv(G[:, j0 * H + h0:], [(H, cnt), (1, rows)]),
                    _sv(F1[:, coff + we:], [(de, cnt), (rowlen, rows)]),
                    _sv(F1[:, coff + ws:], [(ds, cnt), (rowlen, rows)]),
                )
            jbase += s
    nc.vector.memset(F2[:, 0:1], 0.0)
    nc.vector.tensor_tensor_scan(F2[:, 1:1 + NB * H], G[:, :NB * H],
                                 _sv(F2[:, 0:1], [(0, NB * H)]), 0.0, add, byp)
    for s in POOL_SIZES:
        jb = JBASE[s]
        hb = HBINS[s]
        for (o0, cnt, ds, de) in _uniform_runs(hb):
            hs, he = hb[o0]
            nc.vector.tensor_sub(
                _sv(P_out[:, B280[s] + o0 * s:], [(s, cnt), (1, s)]),
                _sv(F2[:, jb * H + he:], [(de, cnt), (H, s)]),
                _sv(F2[:, jb * H + hs:], [(ds, cnt), (H, s)]),
            )


def _emit_kpool(nc, D, kq, F1, G, F2):
    """K-side: kq [(third,kk)=96, TH*W=3072] bf16 -> D [96, 280] f32 raw
    third-local bin sums.  G layout [96, NB*TH] flat j*32+h."""
    add, byp = mybir.AluOpType.add, mybir.AluOpType.bypass
    n = TH * W
    nc.vector.memset(F1[:96, 0:1], 0.0)
    nc.vector.tensor_tensor_scan(F1[:96, 1:1 + n], kq[:, :],
                                 _sv(F1[:96, 0:1], [(0, n)]), 0.0, add, byp)
    jbase = 0
    for s in POOL_SIZES:
        sb = _pool_bins(W, s)
        for (i0, cnt, ds, de) in _uniform_runs(sb):
            ws, we = sb[i0]
            j0 = jbase + i0
            nc.vector.tensor_sub(
                _sv(G[:96, j0 * TH:], [(TH, cnt), (1, TH)]),
                _sv(F1[:96, we:], [(de, cnt), (W, TH)]),
                _sv(F1[:96, ws:], [(ds, cnt), (W, TH)]),
            )
        jbase += s
    nc.vector.memset(F2[:96, 0:1], 0.0)
    nc.vector.tensor_tensor_scan(F2[:96, 1:1 + NB * TH], G[:96, :NB * TH],
                                 _sv(F2[:96, 0:1], [(0, NB * TH)]), 0.0, add, byp)
    nc.vector.memset(D[:, :], 0.0)
    # third-local clipped H-bin diffs, per third partition group
    for t in range(3):
        for s in POOL_SIZES:
            jb = JBASE[s]
            # clipped (lhs, lhe) per output bin o
            cb = []
            for o, (hs, he) in enumerate(HBINS[s]):
                lhs = min(max(hs - TH * t, 0), TH)
                lhe = min(max(he - TH * t, 0), TH)
                cb.append((o, lhs, lhe))
            cb = [(o, a, b) for (o, a, b) in cb if b > a]
            # one instruction per clipped bin (multi-bin strided runs mis-lower)
            for (o0, a0, b0) in cb:
                nc.vector.tensor_sub(
                    _sv(D[32 * t:32 * t + 32, B280[s] + o0 * s:], [(1, s)]),
                    _sv(F2[32 * t:32 * t + 32, jb * TH + b0:], [(TH, s)]),
                    _sv(F2[32 * t:32 * t + 32, jb * TH + a0:], [(TH, s)]),
                )


def _fix_ldweights_waits(nc):
    """Move waits that gate weight data from InstMatmult to its InstLdweights
    (prevents stale-weight races after Tile's 2-byte matmul split)."""
    import copy
    for f in nc.m.functions:
        for bb in f.blocks:
            insts = bb.instructions
            new_list = []
            i = 0
            while i < len(insts):
                inst = insts[i]
                nxt = insts[i + 1] if i + 1 < len(insts) else None
                if (type(inst).__name__ == "InstLdweights" and nxt is not None
                        and type(nxt).__name__ == "InstMatmult"):
                    wl = list(inst.sync_info.on_wait) if inst.sync_info and inst.sync_info.on_wait else []
                    wm = list(nxt.sync_info.on_wait) if nxt.sync_info and nxt.sync_info.on_wait else []
                    waits = wl + wm
                    mm_upd = list(nxt.sync_info.on_update) if nxt.sync_info and nxt.sync_info.on_update else []
                    ld_upd = list(inst.sync_info.on_update) if inst.sync_info and inst.sync_info.on_update else []
                    if len(waits) > 1:
                        for k, w in enumerate(waits[:-1]):
                            pre = copy.deepcopy(inst)
                            pre.name = f"{inst.name}-ldw{k}"
                            pre.sync_info = bass_rust.SyncInfo(on_wait=[w], on_update=[])
                            new_list.append(pre)
                        inst.sync_info = bass_rust.SyncInfo(on_wait=[waits[-1]], on_update=ld_upd)
                        nxt.sync_info = bass_rust.SyncInfo(on_wait=[], on_update=mm_upd)
                    elif len(waits) == 1:
                        inst.sync_info = bass_rust.SyncInfo(on_wait=[waits[0]], on_update=ld_upd)
                        nxt.sync_info = bass_rust.SyncInfo(on_wait=[], on_update=mm_upd)
                    new_list.append(inst)
                    new_list.append(nxt)
                    i += 2
                    continue
                new_list.append(inst)
                i += 1
            bb.instructions[:] = new_list


def build_kernel(split_ctrl=True):
    nc = bacc.Bacc("TRN2", target_bir_lowering=False, debug=False)

    x1 = nc.dram_tensor("x1", [2, 128, HW], BF16, kind="ExternalInput")
    xx = nc.dram_tensor("xx", [2, 128, HW], BF16, kind="ExternalInput")
    wqT = nc.dram_tensor("wqT", [2, 128, C], F32, kind="ExternalInput")
    wkTb = nc.dram_tensor("wkTb", [2, 128, C8], BF16, kind="ExternalInput")
    bqb = nc.dram_tensor("bqb", [128, C], F32, kind="ExternalInput")
    bkb = nc.dram_tensor("bkb", [128, C8], F32, kind="ExternalInput")
    conz = nc.dram_tensor("conz", [6, 128, 96], BF16, kind="ExternalInput")
    ainv = nc.dram_tensor("ainv", [128, NP], F32, kind="ExternalInput")
    aiT3 = nc.dram_tensor("aiT3", [3, 128, C8], F32, kind="ExternalInput")
    identh = nc.dram_tensor("identh", [128, 128], F32, kind="ExternalInput")
    out = nc.dram_tensor("out", [2, 128, HW], BF16, kind="ExternalOutput")

    with tile.TileContext(nc) as tc:
        with (
            tc.tile_pool(name="consts", bufs=1) as cpool,
            tc.tile_pool(name="xpool", bufs=1) as xpool,
            tc.tile_pool(name="scratch", bufs=1) as spool,
            tc.tile_pool(name="ostage", bufs=4) as opool,
        ):
            # ---- constants ----
            wq_t = [cpool.tile([128, C], F32, tag=f"wq{i}", name=f"wq{i}") for i in range(2)]
            wk_t = [cpool.tile([128, C8], BF16, tag=f"wk{i}", name=f"wk{i}") for i in range(2)]
            bq_t = cpool.tile([128, C], F32, tag="bq", name="bq")
            bk_t = cpool.tile([128, C8], F32, tag="bk", name="bk")
            ai_t = cpool.tile([128, NP], F32, tag="ai", name="ai")
            ai3_t = [cpool.tile([128, C8], F32, tag=f"ai3{i}", name=f"ai3{i}") for i in range(3)]
            cz_t = [cpool.tile([128, 96], BF16, tag=f"cz{i}", name=f"cz{i}") for i in range(6)]
            id_t = cpool.tile([128, 128], F32, tag="idt", name="idt")
            # ---- x_1 into zero-padded [128, 98, 98] tiles (conv rhs + q pooling) ----
            xpad = [xpool.tile([128, H + 2, W + 2], BF16, tag=f"xpad{i}", name=f"xpad{i}") for i in range(2)]
            for cc in range(2):
                nc.vector.memset(xpad[cc][:, 0:H + 2:H + 1, :], 0.0)
                nc.vector.memset(xpad[cc][:, 1:H + 1, 0:W + 2:W + 1], 0.0)
            x1v = [x1.ap()[cc].rearrange("p (h w) -> p h w", w=W) for cc in range(2)]
            xxs = [xpool.tile([128, HW], BF16, tag=f"xxs{i}", name=f"xxs{i}") for i in range(2)]
            # load order: xpad c0 (unblocks q pooling), x halves (k-proj),
            # conv/proj weights, xpad c1, remaining consts
            nc.sync.dma_start(xpad[0][:, 1:1 + H, 1:1 + W], x1v[0][:, :, :])
            for cc in range(2):
                for hh in range(2):
                    nc.sync.dma_start(xxs[cc][:, hh * (HW // 2):(hh + 1) * (HW // 2)],
                                      xx.ap()[cc][:, hh * (HW // 2):(hh + 1) * (HW // 2)])
            for i in range(2):
                nc.sync.dma_start(wk_t[i][:], wkTb.ap()[i])
            for i in range(6):
                nc.sync.dma_start(cz_t[i][:], conz.ap()[i])
            nc.sync.dma_start(xpad[1][:, 1:1 + H, 1:1 + W], x1v[1][:, :, :])
            for i in range(2):
                nc.sync.dma_start(wq_t[i][:], wqT.ap()[i])
            nc.sync.dma_start(bq_t[:], bqb.ap())
            nc.sync.dma_start(bk_t[:], bkb.ap())
            nc.sync.dma_start(ai_t[:], ainv.ap())
            for i in range(3):
                nc.sync.dma_start(ai3_t[i][:], aiT3.ap()[i])
            nc.sync.dma_start(id_t[:], identh.ap())

            # ---- scratch ----
            F1 = spool.tile([128, STRIP * (W + 2) + 1], F32, tag="F1", name="F1")
            G = spool.tile([128, NB * H], F32, tag="G", name="G")
            F2 = spool.tile([128, NB * H + 1], F32, tag="F2", name="F2")
            Pq = [cpool.tile([128, NP], F32, tag=f"Pq{i}", name=f"Pq{i}") for i in range(2)]
            kq = spool.tile([96, TH * W], BF16, tag="kq", name="kq")
            Dk = spool.tile([96, NP], F32, tag="Dk", name="Dk")
            Dsh = [spool.tile([32, NP], F32, tag=f"Dsh{t}", name=f"Dsh{t}") for t in range(2)]

            # ---- k = wk @ x, repacked into thirds via tile_position ----
            KCH = 512
            with tc.tile_pool(name="pk", bufs=2, space="PSUM") as pkp:
                for ch in range(HW // KCH):
                    t = ch // 6
                    off = (ch % 6) * KCH
                    kp = pkp.tile([128, KCH], F32, tag="kp", name="kp")
                    for cc in range(2):
                        nc.tensor.matmul(kp[32 * t:32 * t + 32, :], wk_t[cc][:],
                                         xxs[cc][:, ch * KCH:(ch + 1) * KCH],
                                         start=(cc == 0), stop=(cc == 1),
                                         tile_position=(0, 32 * t))
                    nc.scalar.copy(kq[32 * t:32 * t + 32, off:off + KCH],
                                   kp[32 * t:32 * t + 32, :])

            # ---- zpart conv (independent of pooling: overlaps it on PE) ----
            zs = spool.tile([96, HW], BF16, tag="zs", name="zs")
            nc.vector.memset(zs[0:32, 0:W], 0.0)
            nc.vector.memset(zs[64:96, (H - 1) * W:HW], 0.0)
            with tc.tile_pool(name="pz", bufs=3, space="PSUM") as pzp:
                for j in range(NCH):
                    r0 = j * ROWS
                    zp = pzp.tile([96, ROWS * W], F32, tag="zp", name="zp")
                    ti = 0
                    for dx in range(3):
                        for cinc in range(2):
                            rhs = xpad[cinc][:, 1 + r0:1 + r0 + ROWS, dx:dx + W]
                            nc.tensor.matmul(zp[:, :], cz_t[dx * 2 + cinc][:], rhs,
                                             start=(ti == 0), stop=(ti == 5))
                            ti += 1
                    nc.scalar.copy(zs[32:64, r0 * W:(r0 + ROWS) * W], zp[32:64, :])
                    if r0 + ROWS < H:
                        nc.scalar.copy(zs[0:32, (r0 + 1) * W:(r0 + ROWS + 1) * W], zp[0:32, :])
                    else:
                        nc.scalar.copy(zs[0:32, (r0 + 1) * W:HW], zp[0:32, :(ROWS - 1) * W])
                    if r0 == 0:
                        nc.scalar.copy(zs[64:96, 0:(ROWS - 1) * W], zp[64:96, W:])
                    else:
                        nc.scalar.copy(zs[64:96, (r0 - 1) * W:(r0 + ROWS - 1) * W], zp[64:96, :])

            # ---- pooling: q side (2 chunks on x_1), k side (thirds) ----
            xpflat = [xpad[cc][:].rearrange("p h w -> p (h w)") for cc in range(2)]
            for cc in range(2):
                def src_x1(strip, _cc=cc):
                    return (xpflat[_cc][:, (W + 2):(1 + H) * (W + 2)], W + 2, 1)
                _emit_psp(nc, Pq[cc], src_x1, F1, G, F2)
            _emit_kpool(nc, Dk, kq, F1, G, F2)
            # matmul operands must sit at partition base 0: shift thirds 1,2 down
            for t in range(2):
                nc.sync.dma_start(Dsh[t][:, :], Dk[32 * (t + 1):32 * (t + 2), :])

            for cc in range(2):
                nc.vector.tensor_mul(Pq[cc][:], Pq[cc][:], ai_t[:])

            # ---- projections / affinity (affT96 = sigmoid(logits) replicated 3x) ----
            PCH = [(0, 128), (128, 128), (256, 24)]
            pqT = [cpool.tile([n, C], F32, tag=f"pqT{i}", name=f"pqT{i}") for i, (_, n) in enumerate(PCH)]
            pkR = [cpool.tile([n, 96], F32, tag=f"pkR{i}", name=f"pkR{i}") for i, (_, n) in enumerate(PCH)]
            mk1 = cpool.tile([128, C8], F32, tag="mk1", name="mk1")
            affT96 = cpool.tile([96, C], BF16, tag="affT96", name="affT96")

            with tc.tile_pool(name="psmall", bufs=2, space="PSUM") as pps:
                for i, (p0, n) in enumerate(PCH):
                    ps = pps.tile([n, C], F32, tag="ps", name="ps")
                    for cc in range(2):
                        nc.tensor.matmul(ps[:], Pq[cc][:, p0:p0 + n], wq_t[cc][:],
                                         start=(cc == 0), stop=(cc == 1))
                    nc.vector.tensor_add(pqT[i][:], ps[:], bq_t[:n, :])
                for i, (p0, n) in enumerate(PCH):
                    # pkT[p, kk] = sum_t Dk[(t,kk), p] via identity matmuls
                    ps2 = pps.tile([n, C8], F32, tag="ps2", name="ps2")
                    dsrc = [Dk, Dsh[0], Dsh[1]]
                    for t in range(3):
                        nc.tensor.matmul(ps2[:], dsrc[t][0:32, p0:p0 + n],
                                         id_t[0:32, 0:32],
                                         start=(t == 0), stop=(t == 2))
                    nc.vector.tensor_mul(mk1[:n, :], ps2[:], ai3_t[i][:n, :])
                    for g in range(3):
                        nc.vector.tensor_add(pkR[i][:, 32 * g:32 * g + 32],
                                             mk1[:n, :], bk_t[:n, :])
                pa = pps.tile([96, C], F32, tag="pa", name="pa")
                for i in range(3):
                    nc.tensor.matmul(pa[:], pkR[i][:], pqT[i][:],
                                     start=(i == 0), stop=(i == 2))
                nc.scalar.activation(affT96[:], pa[:], mybir.ActivationFunctionType.Sigmoid)

            # ---- out matmuls ----
            with tc.tile_pool(name="po", bufs=3, space="PSUM") as pop:
                for j in range(NCH):
                    r0 = j * ROWS
                    for coutc in range(2):
                        op = pop.tile([128, ROWS * W], F32, tag="op", name="op")
                        nc.tensor.matmul(op[:], affT96[:, coutc * 128:(coutc + 1) * 128],
                                         zs[:, r0 * W:(r0 + ROWS) * W],
                                         start=True, stop=True)
                        ot = opool.tile([128, ROWS * W], BF16, tag="ot", name="ot")
                        if (j + coutc) % 2 == 0:
                            nc.scalar.copy(ot[:], op[:])
                        else:
                            nc.vector.tensor_copy(ot[:], op[:])
                        nc.sync.dma_start(out.ap()[coutc][:, r0 * W:(r0 + ROWS) * W], ot[:])

    if split_ctrl:
        nc.compile()
        _fix_ldweights_waits(nc)
    return nc


_NC_CACHE = {}


def _get_nc():
    if "nc" not in _NC_CACHE:
        _NC_CACHE["nc"] = build_kernel()
    return _NC_CACHE["nc"]


def _conv_cast(x):
    import ml_dtypes
    return np.ascontiguousarray(x, np.float32).astype(ml_dtypes.bfloat16)


def kernel(x_1, x, wq, bq, wk, bk, con):
    import ml_dtypes
    x_1 = _conv_cast(x_1)
    x = _conv_cast(x)
    con = np.asarray(con, np.float32)
    wq = np.asarray(wq, np.float32)
    bq = np.asarray(bq, np.float32)
    wk = np.asarray(wk, np.float32)
    bk = np.asarray(bk, np.float32)

    wqT_h = np.ascontiguousarray(wq.T).reshape(2, 128, C)
    wkTb_h = np.ascontiguousarray(wk.T).reshape(2, 128, C8).astype(ml_dtypes.bfloat16)
    bqb_h = np.tile(bq, (128, 1))
    bkb_h = np.tile(bk, (128, 1))
    # conz[dx*2+cinc, cin, dy*32+kk] = con[kk, cinc*128+cin, dy, dx]
    conz_h = np.ascontiguousarray(
        con.transpose(3, 1, 2, 0)          # [dx, cin256, dy, kk]
        .reshape(3, 2, 128, 3 * C8)
        .transpose(0, 1, 2, 3)
        .reshape(6, 128, 96)
    ).astype(ml_dtypes.bfloat16)
    ai = _area_inv()
    ainv_h = np.tile(ai, (128, 1))
    aiT3_h = np.zeros((3, 128, C8), np.float32)
    for i, (p0, n) in enumerate([(0, 128), (128, 128), (256, 24)]):
        aiT3_h[i, :n, :] = ai[p0:p0 + n, None]
    ident_h = np.eye(128, dtype=np.float32)

    in_maps = []
    for b in range(B):
        in_maps.append({
            "x1": x_1[b].reshape(2, 128, HW),
            "xx": x[b].reshape(2, 128, HW),
            "wqT": wqT_h, "wkTb": wkTb_h, "bqb": bqb_h, "bkb": bkb_h,
            "conz": conz_h, "ainv": ainv_h, "aiT3": aiT3_h, "identh": ident_h,
        })
    global _last_in_maps
    _last_in_maps = in_maps
    nc = _get_nc()
    res = run_bass_kernel_spmd(nc, in_maps, list(range(B)))
    return np.stack([res.results[b]["out"].astype(np.float32).reshape(C, H, W)
                     for b in range(B)])
